# revision 30
# baseline (speedup 1.0000x reference)
"""Trainium2 Bass kernel for a dense transformer decoder block.

Strategy (8 NeuronCores):
  - Attention tensor-parallel over heads (2 heads/core); activations in
    transposed layout [D, tokens]; per-head outputs exchanged with
    chunked AllToAlls (Ulysses style) so each core computes the full wo
    locally for its own 512 tokens; FFN data-parallel on those tokens.
  - fp8 (e4m3) + DoubleRow (paired 256-deep contraction) for the
    q/k/v projections, rmsnorm stats, softmax exp/denominator and PV
    matmuls of q-tiles j>=1; q-tile j=0 of each batch (short causal
    contexts, where per-key fp8 noise does not average out) stays bf16.
    Scores and wo/FFN matmuls stay bf16.  Weight fp8 copies are
    pre-scaled by 2^9 host-side; the descale folds into the rmsnorm
    reciprocal chain for free.
  - wo computed in two N=256 chunks (A2A pairs 0+1 and 2+3) with wo
    streamed per-output-tile; w1/w2 streamed exactly once with N=512
    matmuls; DMA traffic spread across the sync/scalar HWDGE queues and
    the gpsimd SWDGE queue; x/weight startup loads quartered so the PE
    starts within a few us.
  - Causality is not hardcoded: mask blocks are classified host-side
    into skip / plain / mixed tiles (shipped as constants).
"""

import os
import sys

try:  # the axon sitecustomize usually provides concourse already
    import concourse.bass  # noqa: F401
except ImportError:  # pragma: no cover
    sys.path.insert(0, "/opt/trn_rl_repo")

from contextlib import ExitStack

import ml_dtypes
import numpy as np

import concourse.bacc as bacc
import concourse.tile as tile
from concourse import mybir
from concourse.bass_utils import run_bass_kernel_spmd
from concourse.masks import make_identity

F32 = mybir.dt.float32
BF16 = mybir.dt.bfloat16
F16 = mybir.dt.float16
F8 = mybir.dt.float8e4
DR = mybir.MatmulPerfMode.DoubleRow
N_CORES = 8
P = 128
QW = 512  # q-tile / token-tile width
EPS = 1e-6
AF = mybir.ActivationFunctionType
ALU = mybir.AluOpType
BF16_NP = ml_dtypes.bfloat16
F16_NP = np.float16
F8_NP = ml_dtypes.float8_e4m3
G = 4  # A2A chunks (one per pair of q-tiles)
SL = 128  # tokens per core-slice per chunk
SW = 2.0**9  # fp8 weight pre-scale


def ts(i, w):
    return slice(i * w, (i + 1) * w)


def _classify_mask(mask, S):
    """mask: [S, S] additive (q, k). Returns (table, tiles).
    table[(kt, j)] = 'skip' | (tile_idx_or_None, w_lo) where columns
    [0, w_lo) of this score block are fully masked (skipped) and
    tile_idx is a mask tile to add over [w_lo, QW) (None if plain).
    tiles: list of [128, QW] float32 arrays in scoresT ([k, q]) layout."""
    table = {}
    tiles = []
    keys = {}
    for j in range(S // QW):
        for kt in range(S // P):
            sub = mask[ts(j, QW), ts(kt, P)]  # [q, k]
            if np.all(sub <= -1e8):
                table[(kt, j)] = "skip"
                continue
            colmask = np.all(sub <= -1e8, axis=1)  # fully-masked q columns
            w_lo = int(np.argmin(colmask)) if colmask.any() else 0
            if np.all(sub[w_lo:] == 0.0):
                table[(kt, j)] = (None, w_lo)
            else:
                t = np.ascontiguousarray(sub.T.astype(np.float32))  # [k, q]
                key = t.tobytes()
                if key not in keys:
                    keys[key] = len(tiles)
                    tiles.append(t)
                table[(kt, j)] = (keys[key], w_lo)
        # safety: the first non-skip block must cover all columns, since
        # later blocks only accumulate over their own window
        ks = [k for k in range(S // P) if table[(k, j)] != "skip"]
        if ks and table[(ks[0], j)][1] != 0:
            kt0 = ks[0]
            sub = mask[ts(j, QW), ts(kt0, P)]
            t = np.ascontiguousarray(
                np.where(sub <= -1e8, -1e9, sub).T.astype(np.float32)
            )
            key = t.tobytes()
            if key not in keys:
                keys[key] = len(tiles)
                tiles.append(t)
            table[(kt0, j)] = (keys[key], 0)
    return table, tiles


def build_program(B, S, D, H, HID, mask_table, n_mask):
    HD = 128
    assert D == (D // P) * P and H * HD == D
    HPC = H // N_CORES            # heads per core
    assert HPC * N_CORES == H
    C = D // P                    # contraction chunks over D
    CQ = C // 4                   # chunks per quarter
    S_TILES = S // QW             # q tiles per batch
    KT = S // P                   # k tiles per batch
    HIDC = HID // P               # hidden tiles total (64)
    W1CH = 16                     # w1 stream chunks
    HTPC = HIDC // W1CH           # hid tiles per w1 chunk (4)

    nc = bacc.Bacc(trn_type="TRN2", num_devices=N_CORES)

    # x: j=0 tile per batch in bf16 (quartered), j>=1 tiles in fp8
    xbq = nc.dram_tensor("xbq", [B, 4, P, CQ, QW], BF16, kind="ExternalInput").ap()
    xb8h = nc.dram_tensor(
        "xb8h", [B, S_TILES - 1, P, C, QW], F8, kind="ExternalInput"
    ).ap()
    xres = nc.dram_tensor(
        "xres", [2, P, C, 2 * SL], F16, kind="ExternalInput"
    ).ap()
    # qkv weights: bf16 quartered (for j=0) + fp8 scaled (for j>=1)
    wqq = nc.dram_tensor("wqq", [4, CQ, P, HPC * HD], BF16, kind="ExternalInput").ap()
    wkq = nc.dram_tensor("wkq", [4, CQ, P, HPC * HD], BF16, kind="ExternalInput").ap()
    wvb = nc.dram_tensor("wvb", [C, P, HPC * HD], BF16, kind="ExternalInput").ap()
    wq8 = nc.dram_tensor("wq8", [C, P, HPC * HD], F8, kind="ExternalInput").ap()
    wk8 = nc.dram_tensor("wk8", [C, P, HPC * HD], F8, kind="ExternalInput").ap()
    wv8 = nc.dram_tensor("wv8", [C, P, HPC * HD], F8, kind="ExternalInput").ap()
    wo3h = nc.dram_tensor("wo3h", [C, P, H, P], BF16, kind="ExternalInput").ap()
    w1h = nc.dram_tensor("w1h", [W1CH, P, C, QW], BF16, kind="ExternalInput").ap()
    w2h = nc.dram_tensor("w2h", [C, P, HIDC, P], BF16, kind="ExternalInput").ap()
    mk = None
    if n_mask:
        mk = nc.dram_tensor("mk", [n_mask, P, QW], BF16, kind="ExternalInput").ap()

    # chunks 0-2: 128-token core slices; chunk 3 split in two half A2As
    # (64-token slices) so the last exchange starts right after q-tile j2
    a2a_in = [
        nc.dram_tensor(f"a2a_in{g}", [N_CORES, HPC, P, SL], BF16)
        for g in range(3)
    ] + [
        nc.dram_tensor(f"a2a_in3{s}", [N_CORES, HPC, P, SL // 2], BF16)
        for s in "ab"
    ]
    a2a_out = [
        nc.dram_tensor(f"a2a_out{g}", [N_CORES, HPC, P, SL], BF16)
        for g in range(3)
    ] + [
        nc.dram_tensor(f"a2a_out3{s}", [N_CORES, HPC, P, SL // 2], BF16)
        for s in "ab"
    ]
    out = nc.dram_tensor("out", [C, P, QW], F32, kind="ExternalOutput").ap()

    groups = [list(range(N_CORES))]

    with tile.TileContext(nc) as tc, ExitStack() as ctx:
        const = ctx.enter_context(tc.tile_pool(name="const", bufs=1))
        stats = ctx.enter_context(tc.tile_pool(name="stats", bufs=2))
        sqp = ctx.enter_context(tc.tile_pool(name="sq", bufs=2))
        hfp = ctx.enter_context(tc.tile_pool(name="hf", bufs=1))
        psum = ctx.enter_context(tc.tile_pool(name="psum", bufs=1, space="PSUM"))

        ones_f32 = const.tile([P, P], F32)
        nc.vector.memset(ones_f32[:], 1.0)
        ones = const.tile([P, P], BF16)
        nc.vector.tensor_copy(ones[:], ones_f32[:])
        ones2 = const.tile([P, 2, P], F8)
        nc.vector.tensor_copy(ones2[:, 0, :], ones_f32[:])
        nc.vector.tensor_copy(ones2[:, 1, :], ones_f32[:])
        eps_p1 = const.tile([P, 1], F32)
        nc.vector.memset(eps_p1[:], EPS)
        eps_sw = const.tile([P, 1], F32)
        nc.vector.memset(eps_sw[:], EPS * SW * SW)
        ident = const.tile([P, P], F32)
        make_identity(nc, ident[:])
        ident_b = const.tile([P, P], BF16)
        nc.vector.tensor_copy(ident_b[:], ident[:])

        # h for this core's 512 tokens, assembled chunk by chunk
        hf = hfp.tile([P, C, QW], F16, tag="hf")
        NPRE = 6  # w1 chunks pre-started on token cols 0:256
        up_pre = hfp.tile([P, NPRE * HTPC, QW // 2], BF16, tag="up_pre")

        # ---------------- attention phase ----------------
        with tc.tile_pool(name="wqkv", bufs=1) as wqkvp, \
             tc.tile_pool(name="xa", bufs=2) as xap, \
             tc.tile_pool(name="qkv", bufs=1) as qkvp, \
             tc.tile_pool(name="exp", bufs=3) as expp, \
             tc.tile_pool(name="wos", bufs=2) as wosp, \
             tc.tile_pool(name="xr", bufs=1) as xrp, \
             tc.tile_pool(name="ao", bufs=1) as aop, \
             tc.tile_pool(name="attn", bufs=1) as attp:
            # fp8 weights (whole) on scalar queue; bf16 quarters on sync
            wq8_sb = wqkvp.tile([P, C, HPC * HD], F8, tag="wq8")
            wk8_sb = wqkvp.tile([P, C, HPC * HD], F8, tag="wk8")
            wv8_sb = wqkvp.tile([P, C, HPC * HD], F8, tag="wv8")
            wqb_sb = [wqkvp.tile([P, CQ, HPC * HD], BF16, tag=f"wqb{i}",
                                 name=f"wqb{i}") for i in range(4)]
            wkb_sb = [wqkvp.tile([P, CQ, HPC * HD], BF16, tag=f"wkb{i}",
                                 name=f"wkb{i}") for i in range(4)]
            wvb_sb = wqkvp.tile([P, C, HPC * HD], BF16, tag="wvb")
            mtiles = None

            def emit_a2a(g):
                nc.gpsimd.collective_compute(
                    "AllToAll",
                    ALU.bypass,
                    replica_groups=groups,
                    ins=[a2a_in[g].ap().opt()],
                    outs=[a2a_out[g].ap().opt()],
                )

            def stats_mm(xq, x8):
                """squares on DVE (fp8 out), paired column sums via DR."""
                cs = psum.tile([P, QW], F32, tag="mm", bufs=3)
                for cp in range(C // 2):
                    sq = sqp.tile([P, 2, QW], F8, tag="sq")
                    for i in range(2):
                        c = 2 * cp + i
                        src = xq[c // CQ][:, c % CQ, :] if xq is not None \
                            else x8[:, c, :]
                        nc.vector.tensor_mul(sq[:, i, :], src, src)
                    nc.tensor.matmul(
                        cs[:], ones2[:], sq[:],
                        start=(cp == 0), stop=(cp == C // 2 - 1),
                        perf_mode=DR,
                    )
                return cs

            def stats_fin(cs, scaled):
                """sqrt on ACT + fast recip; `scaled` folds the 1/SW fp8
                weight descale into rinv."""
                rms = stats.tile([P, QW], F32, tag="rms")
                if scaled:
                    nc.scalar.activation(
                        rms[:], cs[:], AF.Sqrt, bias=eps_sw[:],
                        scale=SW * SW / D,
                    )
                else:
                    nc.scalar.activation(
                        rms[:], cs[:], AF.Sqrt, bias=eps_p1[:], scale=1.0 / D
                    )
                rinv = stats.tile([P, QW], F32, tag="rinv")
                nc.vector.reciprocal_approx_fast(rinv[:], rms[:])
                return rinv

            def emit_wo_pair(gp):
                """wo for chunks (2gp, 2gp+1): 256 tokens, streamed wo."""
                ao = aop.tile([P, H, 2 * SL], BF16, tag="ao")
                srcs = [(2 * gp, 0, SL), (2 * gp + 1, SL, SL)] if gp == 0 \
                    else [(2, 0, SL), (3, SL, SL // 2), (4, SL + SL // 2, SL // 2)]
                for gi, c0, w in srcs:
                    nc.sync.dma_start(
                        ao[:, :, c0 : c0 + w],
                        a2a_out[gi].ap().rearrange("s h p t -> p (s h) t"),
                    )
                xr = xrp.tile([P, C, 2 * SL], F16, tag="xr")
                nc.gpsimd.dma_start(xr[:], xres[gp])
                for ot in range(C):
                    wo3 = wosp.tile([P, H, P], BF16, tag="wo3")
                    (nc.gpsimd if ot % 2 else nc.sync).dma_start(
                        wo3[:], wo3h[ot]
                    )
                    po = psum.tile([P, 2 * SL], F32, tag="mm", bufs=3)
                    for oc in range(H):
                        nc.tensor.matmul(
                            po[:],
                            wo3[:, oc, :],
                            ao[:, oc, :],
                            start=(oc == 0),
                            stop=(oc == H - 1),
                        )
                    nc.vector.tensor_add(
                        hf[:, ot, ts(gp, 2 * SL)], xr[:, ot, :], po[:]
                    )

            for b in range(B):
                # prefetch x tiles of this batch; j=0 quartered bf16,
                # interleaved with the bf16 weight quarters they feed
                xq = [
                    xap.tile([P, CQ, QW], BF16, tag=f"xq{i}", name=f"xq{b}_{i}",
                             bufs=1)
                    for i in range(4)
                ]
                for i in range(4):
                    # startup: first x quarter + weight quarters race down
                    # the sync queue; the rest stream on gpsimd
                    (nc.sync if (b == 0 and i == 0) else nc.gpsimd).dma_start(
                        xq[i][:], xbq[b, i]
                    )
                    if b == 0:
                        nc.sync.dma_start(
                            wqb_sb[i][:], wqq[i].rearrange("c p o -> p c o")
                        )
                        nc.sync.dma_start(
                            wkb_sb[i][:], wkq[i].rearrange("c p o -> p c o")
                        )
                if b == 0:
                    nc.gpsimd.dma_start(
                        wvb_sb[:], wvb.rearrange("c p o -> p c o")
                    )
                    nc.scalar.dma_start(
                        wq8_sb[:], wq8.rearrange("c p o -> p c o")
                    )
                    nc.scalar.dma_start(
                        wk8_sb[:], wk8.rearrange("c p o -> p c o")
                    )
                    nc.scalar.dma_start(
                        wv8_sb[:], wv8.rearrange("c p o -> p c o")
                    )
                    if n_mask:
                        mtiles = wqkvp.tile([P, n_mask, QW], BF16, tag="mk")
                        nc.scalar.dma_start(
                            mtiles[:], mk.rearrange("n p q -> p n q")
                        )
                x8s = [None]
                for j in range(1, S_TILES):
                    x8 = xap.tile([P, C, QW], F8, tag="xb8", name=f"x8{b}_{j}")
                    (nc.scalar if b == 0 else nc.gpsimd).dma_start(
                        x8[:], xb8h[b, j - 1]
                    )
                    x8s.append(x8)

                rinv_next = stats_fin(stats_mm(xq, None), scaled=False)

                kT = qkvp.tile([P, HPC, S], BF16, tag="kT")
                vN8 = qkvp.tile([P, KT, HPC * HD], F8, tag="vN8")
                vNb = qkvp.tile([P, 4, HPC * HD], BF16, tag="vNb")
                for j in range(S_TILES):
                    g = 2 * b + j // 2
                    r = j % 2
                    f8 = j > 0
                    rinv = rinv_next
                    # ---- q/k projections (rinv folded in at eviction) ----
                    qTs = qkvp.tile([P, HPC, QW], BF16, tag="qT", bufs=2)
                    for h in range(HPC):
                        for w8, wbq, dst in (
                            (wq8_sb, wqb_sb, qTs),
                            (wk8_sb, wkb_sb, kT),
                        ):
                            pp = psum.tile([P, QW], F32, tag="mm", bufs=3)
                            if f8:
                                for cp in range(C // 2):
                                    nc.tensor.matmul(
                                        pp[:],
                                        w8[:, 2 * cp : 2 * cp + 2, ts(h, HD)],
                                        x8s[j][:, 2 * cp : 2 * cp + 2, :],
                                        start=(cp == 0),
                                        stop=(cp == C // 2 - 1),
                                        perf_mode=DR,
                                    )
                            else:
                                for c in range(C):
                                    nc.tensor.matmul(
                                        pp[:],
                                        wbq[c // CQ][:, c % CQ, ts(h, HD)],
                                        xq[c // CQ][:, c % CQ, :],
                                        start=(c == 0),
                                        stop=(c == C - 1),
                                    )
                            if dst is qTs:
                                nc.vector.tensor_mul(qTs[:, h, :], pp[:], rinv[:])
                            else:
                                nc.vector.tensor_mul(
                                    kT[:, h, ts(j, QW)], pp[:], rinv[:]
                                )
                    # rinv transposed to token-partition layout for v scaling
                    rcol = stats.tile([P, QW // P], F32, tag="rcol")
                    for sub in range(QW // P):
                        tp = psum.tile([P, P], F32, tag="mm", bufs=3)
                        nc.tensor.transpose(tp[:], rinv[:, ts(sub, P)], ident[:])
                        nc.vector.tensor_copy(rcol[:, sub : sub + 1], tp[:, 0:1])
                    # v in natural (token-partition) layout
                    for sub in range(QW // P):
                        pv = psum.tile([P, QW], F32, tag="mm", bufs=3)
                        if f8:
                            for cp in range(C // 2):
                                nc.tensor.matmul(
                                    pv[:, : HPC * HD],
                                    x8s[j][:, 2 * cp : 2 * cp + 2, ts(sub, P)],
                                    wv8_sb[:, 2 * cp : 2 * cp + 2, :],
                                    start=(cp == 0),
                                    stop=(cp == C // 2 - 1),
                                    perf_mode=DR,
                                )
                        else:
                            for c in range(C):
                                nc.tensor.matmul(
                                    pv[:, : HPC * HD],
                                    xq[c // CQ][:, c % CQ, ts(sub, P)],
                                    wvb_sb[:, c, :],
                                    start=(c == 0),
                                    stop=(c == C - 1),
                                )
                        kt_i = j * (QW // P) + sub
                        nc.vector.tensor_scalar_mul(
                            vN8[:, kt_i, :],
                            pv[:, : HPC * HD],
                            rcol[:, sub : sub + 1],
                        )
                        if j == 0:
                            nc.vector.tensor_scalar_mul(
                                vNb[:, sub, :],
                                pv[:, : HPC * HD],
                                rcol[:, sub : sub + 1],
                            )

                    # stats matmuls for the NEXT q-tile
                    cs_next = (
                        stats_mm(None, x8s[j + 1]) if j + 1 < S_TILES else None
                    )

                    # -------- attention for q-tile j --------
                    attnT = attp.tile([P, HPC, QW], BF16, tag="attnT", bufs=2)
                    for h in range(HPC):
                        kts = [
                            kt for kt in range(KT) if mask_table[(kt, j)] != "skip"
                        ]
                        pa = psum.tile([P, QW], F32, tag="pv", bufs=1)
                        den = psum.tile([P, QW], F32, tag="stat", bufs=1)

                        def msc_for(kt, w0):
                            """scores (+mask) for block kt over columns
                            [w0, QW); w0 <= this block's own window."""
                            msc = psum.tile([P, QW], F32, tag="score", bufs=3)
                            mt, _wl = mask_table[(kt, j)]
                            if mt is not None:
                                nc.tensor.matmul(
                                    msc[:, w0:], ident_b[:],
                                    mtiles[:, mt, w0:],
                                    start=True, stop=False,
                                )
                            nc.tensor.matmul(
                                msc[:, w0:],
                                kT[:, h, ts(kt, P)],
                                qTs[:, h, w0:],
                                start=(mt is None),
                                stop=True,
                            )
                            return msc

                        wlo = {kt: mask_table[(kt, j)][1] for kt in kts}
                        if f8:
                            # pairs of k-tiles (DoubleRow) + optional tail
                            n_pair = len(kts) // 2
                            odd = len(kts) % 2
                            n_step = n_pair + odd
                            exs = [None] * n_step
                            pw = [
                                min(wlo[kts[2 * i]], wlo[kts[2 * i + 1]])
                                if i < n_pair else wlo[kts[-1]]
                                for i in range(n_step)
                            ]

                            def _den_pv8(i):
                                last = i == n_step - 1
                                w0 = pw[i]
                                if i < n_pair:
                                    k0 = kts[2 * i]
                                    assert kts[2 * i + 1] == k0 + 1
                                    nc.tensor.matmul(
                                        den[:, w0:], ones2[:],
                                        exs[i][:, :, w0:],
                                        start=(i == 0), stop=last,
                                        perf_mode=DR,
                                    )
                                    nc.tensor.matmul(
                                        pa[:, w0:],
                                        vN8[:, k0 : k0 + 2, ts(h, HD)],
                                        exs[i][:, :, w0:],
                                        start=(i == 0),
                                        stop=last,
                                        perf_mode=DR,
                                    )
                                else:  # odd tail, plain fp8 matmul
                                    kt = kts[-1]
                                    nc.tensor.matmul(
                                        den[:, w0:], ones2[:, 0, :],
                                        exs[i][:, 0, w0:],
                                        start=(i == 0), stop=last,
                                    )
                                    nc.tensor.matmul(
                                        pa[:, w0:],
                                        vN8[:, kt, ts(h, HD)],
                                        exs[i][:, 0, w0:],
                                        start=(i == 0),
                                        stop=last,
                                    )

                            for i in range(n_step):
                                ex = expp.tile([P, 2, QW], F8, tag="exp8")
                                for u in range(2 if i < n_pair else 1):
                                    msc = msc_for(kts[2 * i + u], pw[i])
                                    nc.scalar.activation(
                                        ex[:, u, pw[i] :], msc[:, pw[i] :],
                                        AF.Exp,
                                    )
                                exs[i] = ex
                                if i > 0:
                                    _den_pv8(i - 1)
                            _den_pv8(n_step - 1)
                        else:
                            n_k = len(kts)
                            exs = [None] * n_k

                            def _den_pv(i):
                                kt = kts[i]
                                w0 = wlo[kt]
                                nc.tensor.matmul(
                                    den[:, w0:], ones[:], exs[i][:, w0:],
                                    start=(i == 0), stop=(i == n_k - 1),
                                )
                                nc.tensor.matmul(
                                    pa[:, w0:],
                                    vNb[:, kt, ts(h, HD)],
                                    exs[i][:, w0:],
                                    start=(i == 0),
                                    stop=(i == n_k - 1),
                                )

                            for i, kt in enumerate(kts):
                                msc = msc_for(kt, wlo[kt])
                                ex = expp.tile([P, QW], BF16, tag="expb")
                                nc.scalar.activation(
                                    ex[:, wlo[kt] :], msc[:, wlo[kt] :], AF.Exp
                                )
                                exs[i] = ex
                                if i > 0:
                                    _den_pv(i - 1)
                            _den_pv(n_k - 1)
                        rec = stats.tile([P, QW], F32, tag="rec")
                        nc.vector.reciprocal_approx_fast(rec[:], den[:])
                        nc.vector.tensor_mul(attnT[:, h, :], pa[:], rec[:])
                        # ship this head's block immediately
                        if b == 1 and j >= 2:
                            nc.sync.dma_start(
                                a2a_in[1 + j].ap()[:, h, :, :].rearrange(
                                    "s p t -> p s t"
                                ),
                                attnT[:, h, :],
                            )
                        else:
                            nc.sync.dma_start(
                                a2a_in[g].ap()[
                                    4 * r : 4 * r + 4, h, :, :
                                ].rearrange("s p t -> p s t"),
                                attnT[:, h, :],
                            )

                    if cs_next is not None:
                        rinv_next = stats_fin(cs_next, scaled=True)

                    if b == 1 and j == 1:
                        # wo for chunks 0+1, two q-tiles after A2A-1 issue
                        emit_wo_pair(0)

                    if b == 1 and j >= 2:
                        emit_a2a(1 + j)
                    elif r == 1:
                        emit_a2a(g)

            # ---- pre-start up on cols 0:256 (ready after wo pair 0) to
            # fill the final-A2A wait with PE work ----
            with tc.tile_pool(name="w1pre", bufs=2) as w1prep:
                for ch in range(NPRE):
                    w1c = w1prep.tile(
                        [P, C, QW], BF16, tag="w1pre", name=f"w1pre{ch}"
                    )
                    (nc.gpsimd if ch % 2 else nc.sync).dma_start(w1c[:], w1h[ch])
                    for hti in range(HTPC):
                        pu = psum.tile([P, QW // 2], F32, tag="mm", bufs=3)
                        for c in range(C):
                            nc.tensor.matmul(
                                pu[:],
                                w1c[:, c, ts(hti, P)],
                                hf[:, c, 0 : QW // 2],
                                start=(c == 0),
                                stop=(c == C - 1),
                            )
                        # relu on DVE, not ACT: ACT is the attention exp
                        # critical path while this filler work runs
                        nc.vector.tensor_scalar_max(
                            up_pre[:, ch * HTPC + hti, :], pu[:], 0.0
                        )

                # ---- wo for chunks 2+3 (tail, after A2A-3a/3b) ----
                emit_wo_pair(1)

        # ---------------- FFN phase (data-parallel, 512 tokens/core) --------
        with tc.tile_pool(name="w1p", bufs=2) as w1p, \
             tc.tile_pool(name="w2p", bufs=3) as w2p, \
             tc.tile_pool(name="up", bufs=1) as upp, \
             tc.tile_pool(name="oev", bufs=3) as oevp:
            up_sb = upp.tile([P, HIDC, QW], BF16, tag="up")

            # rmsnorm2 stats (consumed only at the down outputs)
            cs = psum.tile([P, QW], F32, tag="mm", bufs=3)
            for cp in range(C // 2):
                sq = sqp.tile([P, 2, QW], F8, tag="sq")
                for i in range(2):
                    nc.vector.tensor_mul(
                        sq[:, i, :], hf[:, 2 * cp + i, :], hf[:, 2 * cp + i, :]
                    )
                nc.tensor.matmul(
                    cs[:], ones2[:], sq[:],
                    start=(cp == 0), stop=(cp == C // 2 - 1),
                    perf_mode=DR,
                )
            rms2 = stats.tile([P, QW], F32, tag="rms")
            nc.scalar.activation(
                rms2[:], cs[:], AF.Sqrt, bias=eps_p1[:], scale=1.0 / D
            )
            r2 = stats.tile([P, QW], F32, tag="rinv")
            nc.vector.reciprocal_approx_fast(r2[:], rms2[:])

            # pre-started halves: copy into up_sb
            for idx in range(NPRE * HTPC):
                nc.vector.tensor_copy(
                    up_sb[:, idx, 0 : QW // 2], up_pre[:, idx, :]
                )

            # ---- up: single w1 stream, N=512 (relu defers rinv2);
            # pre-started chunks only need their remaining columns ----
            for ch in range(W1CH):
                w1c = w1p.tile([P, C, QW], BF16, tag="w1", name=f"w1_{ch}")
                (nc.scalar if ch % 2 else nc.sync).dma_start(w1c[:], w1h[ch])
                cols = slice(QW // 2, QW) if ch < NPRE else slice(0, QW)
                for hti in range(HTPC):
                    pu = psum.tile([P, QW], F32, tag="mm", bufs=3)
                    pw = cols.stop - cols.start
                    for c in range(C):
                        nc.tensor.matmul(
                            pu[:, :pw],
                            w1c[:, c, ts(hti, P)],
                            hf[:, c, cols],
                            start=(c == 0),
                            stop=(c == C - 1),
                        )
                    nc.scalar.activation(
                        up_sb[:, ch * HTPC + hti, cols], pu[:, :pw], AF.Relu
                    )

            # ---- down ----
            for ot in range(C):
                w2c = w2p.tile([P, HIDC, P], BF16, tag="w2", name=f"w2_{ot}")
                (nc.scalar if ot % 2 else nc.sync).dma_start(w2c[:], w2h[ot])
                pd = psum.tile([P, QW], F32, tag="mm", bufs=3)
                for hc in range(HIDC):
                    nc.tensor.matmul(
                        pd[:],
                        w2c[:, hc, :],
                        up_sb[:, hc, :],
                        start=(hc == 0),
                        stop=(hc == HIDC - 1),
                    )
                dn = oevp.tile([P, QW], F32, tag="dn")
                nc.vector.tensor_mul(dn[:], pd[:], r2[:])
                oev = oevp.tile([P, QW], F32, tag="oev")
                nc.vector.tensor_add(oev[:], hf[:, ot, :], dn[:])
                (nc.scalar if ot % 2 else nc.sync).dma_start(out[ot], oev[:])

    nc.compile()
    return nc


_CACHE = {}
LAST_RESULT = None


def _get_program(B, S, D, H, HID, mask_table, n_mask, mask_key):
    key = (B, S, D, H, HID, mask_key)
    if key not in _CACHE:
        _CACHE[key] = build_program(B, S, D, H, HID, mask_table, n_mask)
    return _CACHE[key]


def _core_tokens(core, S):
    """(token start, length) of this core's output slices, in the order
    they appear in the per-core 512 token columns. Chunks 0-2 are
    128-token slices; the split last A2A gives two 64-token slices."""
    toks = []
    for g in range(3):
        b = g // 2
        j = 2 * (g % 2) + core // 4
        toks.append((S * b + QW * j + SL * (core % 4), SL))
    toks.append((S + QW * 2 + (SL // 2) * core, SL // 2))
    toks.append((S + QW * 3 + (SL // 2) * core, SL // 2))
    return toks


def kernel(x, mask, wq, wk, wv, wo, w1, w2, attn_norm_w, ffn_norm_w):
    x = np.asarray(x, dtype=np.float32)
    mask = np.asarray(mask, dtype=np.float32)
    wq, wk, wv, wo = (np.asarray(a, dtype=np.float32) for a in (wq, wk, wv, wo))
    w1, w2 = np.asarray(w1, dtype=np.float32), np.asarray(w2, dtype=np.float32)
    attn_norm_w = np.asarray(attn_norm_w, dtype=np.float32)
    ffn_norm_w = np.asarray(ffn_norm_w, dtype=np.float32)

    B, S, D = x.shape
    H = D // 128  # HD is fixed at 128 (= SBUF partition count)
    HID = w1.shape[0]
    HD = D // H
    HPC = H // N_CORES
    C = D // P
    CQ = C // 4
    S_TILES = S // QW
    HIDC = HID // P
    W1CH = 16

    mask_table, mtiles_np = _classify_mask(
        np.broadcast_to(mask, (1, 1, S, S))[0, 0], S
    )
    mask_key = hash(tuple(sorted((k, str(v)) for k, v in mask_table.items())))
    nc = _get_program(B, S, D, H, HID, mask_table, len(mtiles_np), mask_key)

    # ---- host-side prep ----
    # x tiles: [b, j, p, c, t] = x[b, j*QW + t, c*128 + p]
    xt = x.reshape(B, S_TILES, QW, C, P).transpose(0, 1, 4, 3, 2)
    xbq = np.ascontiguousarray(
        xt[:, 0].reshape(B, P, 4, CQ, QW).transpose(0, 2, 1, 3, 4)
    ).astype(BF16_NP)
    xb8h = np.ascontiguousarray(xt[:, 1:]).astype(F8_NP)

    wq_f = (wq * attn_norm_w[None, :]) / np.sqrt(HD)
    wk_f = wk * attn_norm_w[None, :]
    wv_f = wv * attn_norm_w[None, :]
    w1_f = w1 * ffn_norm_w[None, :]

    # wo3h[ot, p, oc, o] = wo[ot*128 + o, oc*128 + p]
    wo3host = np.ascontiguousarray(
        wo.reshape(C, P, H, P).transpose(0, 3, 2, 1)
    ).astype(BF16_NP)
    # w1h[ch, p, c, o] = w1_f[hid = ch*512 + o, d = c*128 + p]
    w1host = np.ascontiguousarray(
        w1_f.reshape(W1CH, QW, C, P).transpose(0, 3, 2, 1)
    ).astype(BF16_NP)
    # w2h[ot, p, hc, o] = w2[d_out = ot*128 + o, hid = hc*128 + p]
    w2host = np.ascontiguousarray(
        w2.reshape(C, P, HIDC, P).transpose(0, 3, 2, 1)
    ).astype(BF16_NP)

    xf = x.reshape(B * S, D)
    in_maps = []
    for core in range(N_CORES):
        hs = slice(core * HPC * HD, (core + 1) * HPC * HD)
        qsb = np.ascontiguousarray(wq_f[hs].T).reshape(C, P, HPC * HD)
        ksb = np.ascontiguousarray(wk_f[hs].T).reshape(C, P, HPC * HD)
        vsb = np.ascontiguousarray(wv_f[hs].T).reshape(C, P, HPC * HD)
        # xres[gp, p, cc, t]: 256 tokens per wo pair
        xr = np.empty((2, P, C, 2 * SL), dtype=np.float32)
        col = 0
        for tok0, ln in _core_tokens(core, S):
            gp, c0 = col // (2 * SL), col % (2 * SL)
            xr[gp, :, :, c0 : c0 + ln] = (
                xf[tok0 : tok0 + ln, :].T.reshape(C, P, ln).transpose(1, 0, 2)
            )
            col += ln
        m = {
            "xbq": xbq,
            "xb8h": xb8h,
            "xres": xr.astype(F16_NP),
            "wqq": qsb.reshape(4, CQ, P, HPC * HD).astype(BF16_NP),
            "wkq": ksb.reshape(4, CQ, P, HPC * HD).astype(BF16_NP),
            "wvb": vsb.astype(BF16_NP),
            "wq8": (qsb * SW).astype(F8_NP),
            "wk8": (ksb * SW).astype(F8_NP),
            "wv8": (vsb * SW).astype(F8_NP),
            "wo3h": wo3host,
            "w1h": w1host,
            "w2h": w2host,
        }
        if len(mtiles_np):
            m["mk"] = np.stack(mtiles_np).astype(BF16_NP)
        in_maps.append(m)

    trace = os.environ.get("KTRACE", "0") == "1"
    res = run_bass_kernel_spmd(nc, in_maps, list(range(N_CORES)), trace=trace)
    global LAST_RESULT
    LAST_RESULT = res

    full = np.empty((B * S, D), dtype=np.float32)
    for core in range(N_CORES):
        o = res.results[core]["out"].reshape(D, QW)
        col = 0
        for tok0, ln in _core_tokens(core, S):
            full[tok0 : tok0 + ln, :] = o[:, col : col + ln].T
            col += ln
    return np.ascontiguousarray(full.reshape(B, S, D))


# revision 31
# speedup vs baseline: 1.0036x; 1.0036x over previous
"""Trainium2 Bass kernel for a dense transformer decoder block.

Strategy (8 NeuronCores):
  - Attention tensor-parallel over heads (2 heads/core); activations in
    transposed layout [D, tokens]; per-head outputs exchanged with
    chunked AllToAlls (Ulysses style) so each core computes the full wo
    locally for its own 512 tokens; FFN data-parallel on those tokens.
  - fp8 (e4m3) + DoubleRow (paired 256-deep contraction) for the
    q/k/v projections, rmsnorm stats, softmax exp/denominator and PV
    matmuls of q-tiles j>=1; q-tile j=0 of each batch (short causal
    contexts, where per-key fp8 noise does not average out) stays bf16.
    Scores and wo/FFN matmuls stay bf16.  Weight fp8 copies are
    pre-scaled by 2^9 host-side; the descale folds into the rmsnorm
    reciprocal chain for free.
  - wo computed in two N=256 chunks (A2A pairs 0+1 and 2+3) with wo
    streamed per-output-tile; w1/w2 streamed exactly once with N=512
    matmuls; DMA traffic spread across the sync/scalar HWDGE queues and
    the gpsimd SWDGE queue; x/weight startup loads quartered so the PE
    starts within a few us.
  - Causality is not hardcoded: mask blocks are classified host-side
    into skip / plain / mixed tiles (shipped as constants).
"""

import os
import sys

try:  # the axon sitecustomize usually provides concourse already
    import concourse.bass  # noqa: F401
except ImportError:  # pragma: no cover
    sys.path.insert(0, "/opt/trn_rl_repo")

from contextlib import ExitStack

import ml_dtypes
import numpy as np

import concourse.bacc as bacc
import concourse.tile as tile
from concourse import mybir
from concourse.bass_utils import run_bass_kernel_spmd
from concourse.masks import make_identity

F32 = mybir.dt.float32
BF16 = mybir.dt.bfloat16
F16 = mybir.dt.float16
F8 = mybir.dt.float8e4
DR = mybir.MatmulPerfMode.DoubleRow
N_CORES = 8
P = 128
QW = 512  # q-tile / token-tile width
EPS = 1e-6
AF = mybir.ActivationFunctionType
ALU = mybir.AluOpType
BF16_NP = ml_dtypes.bfloat16
F16_NP = np.float16
F8_NP = ml_dtypes.float8_e4m3
G = 4  # A2A chunks (one per pair of q-tiles)
SL = 128  # tokens per core-slice per chunk
SW = 2.0**9  # fp8 weight pre-scale


def ts(i, w):
    return slice(i * w, (i + 1) * w)


def _classify_mask(mask, S):
    """mask: [S, S] additive (q, k). Returns (table, tiles).
    table[(kt, j)] = 'skip' | (tile_idx_or_None, w_lo) where columns
    [0, w_lo) of this score block are fully masked (skipped) and
    tile_idx is a mask tile to add over [w_lo, QW) (None if plain).
    tiles: list of [128, QW] float32 arrays in scoresT ([k, q]) layout."""
    table = {}
    tiles = []
    keys = {}
    for j in range(S // QW):
        for kt in range(S // P):
            sub = mask[ts(j, QW), ts(kt, P)]  # [q, k]
            if np.all(sub <= -1e8):
                table[(kt, j)] = "skip"
                continue
            colmask = np.all(sub <= -1e8, axis=1)  # fully-masked q columns
            w_lo = int(np.argmin(colmask)) if colmask.any() else 0
            if np.all(sub[w_lo:] == 0.0):
                table[(kt, j)] = (None, w_lo)
            else:
                t = np.ascontiguousarray(sub.T.astype(np.float32))  # [k, q]
                key = t.tobytes()
                if key not in keys:
                    keys[key] = len(tiles)
                    tiles.append(t)
                table[(kt, j)] = (keys[key], w_lo)
        # safety: the first non-skip block must cover all columns, since
        # later blocks only accumulate over their own window
        ks = [k for k in range(S // P) if table[(k, j)] != "skip"]
        if ks and table[(ks[0], j)][1] != 0:
            kt0 = ks[0]
            sub = mask[ts(j, QW), ts(kt0, P)]
            t = np.ascontiguousarray(
                np.where(sub <= -1e8, -1e9, sub).T.astype(np.float32)
            )
            key = t.tobytes()
            if key not in keys:
                keys[key] = len(tiles)
                tiles.append(t)
            table[(kt0, j)] = (keys[key], 0)
    return table, tiles


def build_program(B, S, D, H, HID, mask_table, n_mask):
    HD = 128
    assert D == (D // P) * P and H * HD == D
    HPC = H // N_CORES            # heads per core
    assert HPC * N_CORES == H
    C = D // P                    # contraction chunks over D
    CQ = C // 4                   # chunks per quarter
    S_TILES = S // QW             # q tiles per batch
    KT = S // P                   # k tiles per batch
    HIDC = HID // P               # hidden tiles total (64)
    W1CH = 16                     # w1 stream chunks
    HTPC = HIDC // W1CH           # hid tiles per w1 chunk (4)

    nc = bacc.Bacc(trn_type="TRN2", num_devices=N_CORES)

    # x: j=0 tile per batch in bf16 (quartered), j>=1 tiles in fp8
    xbq = nc.dram_tensor("xbq", [B, 4, P, CQ, QW], BF16, kind="ExternalInput").ap()
    xb8h = nc.dram_tensor(
        "xb8h", [B, S_TILES - 1, P, C, QW], F8, kind="ExternalInput"
    ).ap()
    xres = nc.dram_tensor(
        "xres", [2, P, C, 2 * SL], F16, kind="ExternalInput"
    ).ap()
    # qkv weights: bf16 quartered (for j=0) + fp8 scaled (for j>=1)
    wqq = nc.dram_tensor("wqq", [4, CQ, P, HPC * HD], BF16, kind="ExternalInput").ap()
    wkq = nc.dram_tensor("wkq", [4, CQ, P, HPC * HD], BF16, kind="ExternalInput").ap()
    wvb = nc.dram_tensor("wvb", [C, P, HPC * HD], BF16, kind="ExternalInput").ap()
    wq8 = nc.dram_tensor("wq8", [C, P, HPC * HD], F8, kind="ExternalInput").ap()
    wk8 = nc.dram_tensor("wk8", [C, P, HPC * HD], F8, kind="ExternalInput").ap()
    wv8 = nc.dram_tensor("wv8", [C, P, HPC * HD], F8, kind="ExternalInput").ap()
    wo3h = nc.dram_tensor("wo3h", [C, P, H, P], BF16, kind="ExternalInput").ap()
    w1h = nc.dram_tensor("w1h", [W1CH, P, C, QW], BF16, kind="ExternalInput").ap()
    w2h = nc.dram_tensor("w2h", [C, P, HIDC, P], BF16, kind="ExternalInput").ap()
    mk = None
    if n_mask:
        mk = nc.dram_tensor("mk", [n_mask, P, QW], BF16, kind="ExternalInput").ap()

    # chunks 0-2: 128-token core slices; chunk 3 split in two half A2As
    # (64-token slices) so the last exchange starts right after q-tile j2
    a2a_in = [
        nc.dram_tensor(f"a2a_in{g}", [N_CORES, HPC, P, SL], BF16)
        for g in range(3)
    ] + [
        nc.dram_tensor(f"a2a_in3{s}", [N_CORES, HPC, P, SL // 2], BF16)
        for s in "ab"
    ]
    a2a_out = [
        nc.dram_tensor(f"a2a_out{g}", [N_CORES, HPC, P, SL], BF16)
        for g in range(3)
    ] + [
        nc.dram_tensor(f"a2a_out3{s}", [N_CORES, HPC, P, SL // 2], BF16)
        for s in "ab"
    ]
    out = nc.dram_tensor("out", [C, P, QW], F32, kind="ExternalOutput").ap()

    groups = [list(range(N_CORES))]

    with tile.TileContext(nc) as tc, ExitStack() as ctx:
        const = ctx.enter_context(tc.tile_pool(name="const", bufs=1))
        stats = ctx.enter_context(tc.tile_pool(name="stats", bufs=2))
        sqp = ctx.enter_context(tc.tile_pool(name="sq", bufs=2))
        hfp = ctx.enter_context(tc.tile_pool(name="hf", bufs=1))
        psum = ctx.enter_context(tc.tile_pool(name="psum", bufs=1, space="PSUM"))

        ones_f32 = const.tile([P, P], F32)
        nc.vector.memset(ones_f32[:], 1.0)
        ones = const.tile([P, P], BF16)
        nc.vector.tensor_copy(ones[:], ones_f32[:])
        ones2 = const.tile([P, 2, P], F8)
        nc.vector.tensor_copy(ones2[:, 0, :], ones_f32[:])
        nc.vector.tensor_copy(ones2[:, 1, :], ones_f32[:])
        eps_p1 = const.tile([P, 1], F32)
        nc.vector.memset(eps_p1[:], EPS)
        eps_sw = const.tile([P, 1], F32)
        nc.vector.memset(eps_sw[:], EPS * SW * SW)
        ident = const.tile([P, P], F32)
        make_identity(nc, ident[:])
        ident_b = const.tile([P, P], BF16)
        nc.vector.tensor_copy(ident_b[:], ident[:])

        # h for this core's 512 tokens, assembled chunk by chunk
        hf = hfp.tile([P, C, QW], F16, tag="hf")
        NPRE = 6  # w1 chunks pre-started on token cols 0:256
        up_pre = hfp.tile([P, NPRE * HTPC, QW // 2], BF16, tag="up_pre")

        # ---------------- attention phase ----------------
        with tc.tile_pool(name="wqkv", bufs=1) as wqkvp, \
             tc.tile_pool(name="xa", bufs=2) as xap, \
             tc.tile_pool(name="qkv", bufs=1) as qkvp, \
             tc.tile_pool(name="exp", bufs=3) as expp, \
             tc.tile_pool(name="wos", bufs=2) as wosp, \
             tc.tile_pool(name="xr", bufs=1) as xrp, \
             tc.tile_pool(name="ao", bufs=1) as aop, \
             tc.tile_pool(name="attn", bufs=1) as attp:
            # fp8 weights (whole) on scalar queue; bf16 quarters on sync
            wq8_sb = wqkvp.tile([P, C, HPC * HD], F8, tag="wq8")
            wk8_sb = wqkvp.tile([P, C, HPC * HD], F8, tag="wk8")
            wv8_sb = wqkvp.tile([P, C, HPC * HD], F8, tag="wv8")
            wqb_sb = [wqkvp.tile([P, CQ, HPC * HD], BF16, tag=f"wqb{i}",
                                 name=f"wqb{i}") for i in range(4)]
            wkb_sb = [wqkvp.tile([P, CQ, HPC * HD], BF16, tag=f"wkb{i}",
                                 name=f"wkb{i}") for i in range(4)]
            wvb_sb = wqkvp.tile([P, C, HPC * HD], BF16, tag="wvb")
            mtiles = None

            def emit_a2a(g):
                nc.gpsimd.collective_compute(
                    "AllToAll",
                    ALU.bypass,
                    replica_groups=groups,
                    ins=[a2a_in[g].ap().opt()],
                    outs=[a2a_out[g].ap().opt()],
                )

            def stats_mm(xq, x8):
                """squares on DVE (fp8 out), paired column sums via DR."""
                cs = psum.tile([P, QW], F32, tag="mm", bufs=3)
                for cp in range(C // 2):
                    sq = sqp.tile([P, 2, QW], F8, tag="sq")
                    for i in range(2):
                        c = 2 * cp + i
                        src = xq[c // CQ][:, c % CQ, :] if xq is not None \
                            else x8[:, c, :]
                        nc.vector.tensor_mul(sq[:, i, :], src, src)
                    nc.tensor.matmul(
                        cs[:], ones2[:], sq[:],
                        start=(cp == 0), stop=(cp == C // 2 - 1),
                        perf_mode=DR,
                    )
                return cs

            def stats_fin(cs, scaled):
                """sqrt on ACT + fast recip; `scaled` folds the 1/SW fp8
                weight descale into rinv."""
                rms = stats.tile([P, QW], F32, tag="rms")
                if scaled:
                    nc.scalar.activation(
                        rms[:], cs[:], AF.Sqrt, bias=eps_sw[:],
                        scale=SW * SW / D,
                    )
                else:
                    nc.scalar.activation(
                        rms[:], cs[:], AF.Sqrt, bias=eps_p1[:], scale=1.0 / D
                    )
                rinv = stats.tile([P, QW], F32, tag="rinv")
                nc.vector.reciprocal_approx_fast(rinv[:], rms[:])
                return rinv

            def emit_wo_pair(gp):
                """wo for chunks (2gp, 2gp+1): 256 tokens, streamed wo."""
                ao = aop.tile([P, H, 2 * SL], BF16, tag="ao")
                srcs = [(2 * gp, 0, SL), (2 * gp + 1, SL, SL)] if gp == 0 \
                    else [(2, 0, SL), (3, SL, SL // 2), (4, SL + SL // 2, SL // 2)]
                for gi, c0, w in srcs:
                    nc.sync.dma_start(
                        ao[:, :, c0 : c0 + w],
                        a2a_out[gi].ap().rearrange("s h p t -> p (s h) t"),
                    )
                xr = xrp.tile([P, C, 2 * SL], F16, tag="xr")
                nc.gpsimd.dma_start(xr[:], xres[gp])
                for ot in range(C):
                    wo3 = wosp.tile([P, H, P], BF16, tag="wo3")
                    (nc.gpsimd if ot % 2 else nc.sync).dma_start(
                        wo3[:], wo3h[ot]
                    )
                    po = psum.tile([P, 2 * SL], F32, tag="mm", bufs=3)
                    for oc in range(H):
                        nc.tensor.matmul(
                            po[:],
                            wo3[:, oc, :],
                            ao[:, oc, :],
                            start=(oc == 0),
                            stop=(oc == H - 1),
                        )
                    nc.vector.tensor_add(
                        hf[:, ot, ts(gp, 2 * SL)], xr[:, ot, :], po[:]
                    )

            for b in range(B):
                # prefetch x tiles of this batch; j=0 quartered bf16,
                # interleaved with the bf16 weight quarters they feed
                xq = [
                    xap.tile([P, CQ, QW], BF16, tag=f"xq{i}", name=f"xq{b}_{i}",
                             bufs=1)
                    for i in range(4)
                ]
                for i in range(4):
                    (nc.sync if b == 0 else nc.gpsimd).dma_start(
                        xq[i][:], xbq[b, i]
                    )
                    if b == 0:
                        nc.sync.dma_start(
                            wqb_sb[i][:], wqq[i].rearrange("c p o -> p c o")
                        )
                        nc.sync.dma_start(
                            wkb_sb[i][:], wkq[i].rearrange("c p o -> p c o")
                        )
                if b == 0:
                    nc.sync.dma_start(
                        wvb_sb[:], wvb.rearrange("c p o -> p c o")
                    )
                    nc.scalar.dma_start(
                        wq8_sb[:], wq8.rearrange("c p o -> p c o")
                    )
                    nc.scalar.dma_start(
                        wk8_sb[:], wk8.rearrange("c p o -> p c o")
                    )
                    nc.scalar.dma_start(
                        wv8_sb[:], wv8.rearrange("c p o -> p c o")
                    )
                    if n_mask:
                        mtiles = wqkvp.tile([P, n_mask, QW], BF16, tag="mk")
                        nc.scalar.dma_start(
                            mtiles[:], mk.rearrange("n p q -> p n q")
                        )
                x8s = [None]
                for j in range(1, S_TILES):
                    x8 = xap.tile([P, C, QW], F8, tag="xb8", name=f"x8{b}_{j}")
                    (nc.scalar if b == 0 else nc.gpsimd).dma_start(
                        x8[:], xb8h[b, j - 1]
                    )
                    x8s.append(x8)

                rinv_next = stats_fin(stats_mm(xq, None), scaled=False)

                kT = qkvp.tile([P, HPC, S], BF16, tag="kT")
                vN8 = qkvp.tile([P, KT, HPC * HD], F8, tag="vN8")
                vNb = qkvp.tile([P, 4, HPC * HD], BF16, tag="vNb")
                for j in range(S_TILES):
                    g = 2 * b + j // 2
                    r = j % 2
                    f8 = j > 0
                    rinv = rinv_next
                    # ---- q/k projections (rinv folded in at eviction) ----
                    qTs = qkvp.tile([P, HPC, QW], BF16, tag="qT", bufs=2)
                    for h in range(HPC):
                        for w8, wbq, dst in (
                            (wq8_sb, wqb_sb, qTs),
                            (wk8_sb, wkb_sb, kT),
                        ):
                            pp = psum.tile([P, QW], F32, tag="mm", bufs=3)
                            if f8:
                                for cp in range(C // 2):
                                    nc.tensor.matmul(
                                        pp[:],
                                        w8[:, 2 * cp : 2 * cp + 2, ts(h, HD)],
                                        x8s[j][:, 2 * cp : 2 * cp + 2, :],
                                        start=(cp == 0),
                                        stop=(cp == C // 2 - 1),
                                        perf_mode=DR,
                                    )
                            else:
                                for c in range(C):
                                    nc.tensor.matmul(
                                        pp[:],
                                        wbq[c // CQ][:, c % CQ, ts(h, HD)],
                                        xq[c // CQ][:, c % CQ, :],
                                        start=(c == 0),
                                        stop=(c == C - 1),
                                    )
                            if dst is qTs:
                                nc.vector.tensor_mul(qTs[:, h, :], pp[:], rinv[:])
                            else:
                                nc.vector.tensor_mul(
                                    kT[:, h, ts(j, QW)], pp[:], rinv[:]
                                )
                    # rinv transposed to token-partition layout for v scaling
                    rcol = stats.tile([P, QW // P], F32, tag="rcol")
                    for sub in range(QW // P):
                        tp = psum.tile([P, P], F32, tag="mm", bufs=3)
                        nc.tensor.transpose(tp[:], rinv[:, ts(sub, P)], ident[:])
                        nc.vector.tensor_copy(rcol[:, sub : sub + 1], tp[:, 0:1])
                    # v in natural (token-partition) layout
                    for sub in range(QW // P):
                        pv = psum.tile([P, QW], F32, tag="mm", bufs=3)
                        if f8:
                            for cp in range(C // 2):
                                nc.tensor.matmul(
                                    pv[:, : HPC * HD],
                                    x8s[j][:, 2 * cp : 2 * cp + 2, ts(sub, P)],
                                    wv8_sb[:, 2 * cp : 2 * cp + 2, :],
                                    start=(cp == 0),
                                    stop=(cp == C // 2 - 1),
                                    perf_mode=DR,
                                )
                        else:
                            for c in range(C):
                                nc.tensor.matmul(
                                    pv[:, : HPC * HD],
                                    xq[c // CQ][:, c % CQ, ts(sub, P)],
                                    wvb_sb[:, c, :],
                                    start=(c == 0),
                                    stop=(c == C - 1),
                                )
                        kt_i = j * (QW // P) + sub
                        nc.vector.tensor_scalar_mul(
                            vN8[:, kt_i, :],
                            pv[:, : HPC * HD],
                            rcol[:, sub : sub + 1],
                        )
                        if j == 0:
                            nc.vector.tensor_scalar_mul(
                                vNb[:, sub, :],
                                pv[:, : HPC * HD],
                                rcol[:, sub : sub + 1],
                            )

                    # stats matmuls for the NEXT q-tile
                    cs_next = (
                        stats_mm(None, x8s[j + 1]) if j + 1 < S_TILES else None
                    )

                    # -------- attention for q-tile j --------
                    attnT = attp.tile([P, HPC, QW], BF16, tag="attnT", bufs=2)
                    for h in range(HPC):
                        kts = [
                            kt for kt in range(KT) if mask_table[(kt, j)] != "skip"
                        ]
                        pa = psum.tile([P, QW], F32, tag="pv", bufs=1)
                        den = psum.tile([P, QW], F32, tag="stat", bufs=1)

                        def msc_for(kt, w0):
                            """scores (+mask) for block kt over columns
                            [w0, QW); w0 <= this block's own window."""
                            msc = psum.tile([P, QW], F32, tag="score", bufs=3)
                            mt, _wl = mask_table[(kt, j)]
                            if mt is not None:
                                nc.tensor.matmul(
                                    msc[:, w0:], ident_b[:],
                                    mtiles[:, mt, w0:],
                                    start=True, stop=False,
                                )
                            nc.tensor.matmul(
                                msc[:, w0:],
                                kT[:, h, ts(kt, P)],
                                qTs[:, h, w0:],
                                start=(mt is None),
                                stop=True,
                            )
                            return msc

                        wlo = {kt: mask_table[(kt, j)][1] for kt in kts}
                        if f8:
                            # pairs of k-tiles (DoubleRow) + optional tail
                            n_pair = len(kts) // 2
                            odd = len(kts) % 2
                            n_step = n_pair + odd
                            exs = [None] * n_step
                            pw = [
                                min(wlo[kts[2 * i]], wlo[kts[2 * i + 1]])
                                if i < n_pair else wlo[kts[-1]]
                                for i in range(n_step)
                            ]

                            def _den_pv8(i):
                                last = i == n_step - 1
                                w0 = pw[i]
                                if i < n_pair:
                                    k0 = kts[2 * i]
                                    assert kts[2 * i + 1] == k0 + 1
                                    nc.tensor.matmul(
                                        den[:, w0:], ones2[:],
                                        exs[i][:, :, w0:],
                                        start=(i == 0), stop=last,
                                        perf_mode=DR,
                                    )
                                    nc.tensor.matmul(
                                        pa[:, w0:],
                                        vN8[:, k0 : k0 + 2, ts(h, HD)],
                                        exs[i][:, :, w0:],
                                        start=(i == 0),
                                        stop=last,
                                        perf_mode=DR,
                                    )
                                else:  # odd tail, plain fp8 matmul
                                    kt = kts[-1]
                                    nc.tensor.matmul(
                                        den[:, w0:], ones2[:, 0, :],
                                        exs[i][:, 0, w0:],
                                        start=(i == 0), stop=last,
                                    )
                                    nc.tensor.matmul(
                                        pa[:, w0:],
                                        vN8[:, kt, ts(h, HD)],
                                        exs[i][:, 0, w0:],
                                        start=(i == 0),
                                        stop=last,
                                    )

                            for i in range(n_step):
                                ex = expp.tile([P, 2, QW], F8, tag="exp8")
                                for u in range(2 if i < n_pair else 1):
                                    msc = msc_for(kts[2 * i + u], pw[i])
                                    nc.scalar.activation(
                                        ex[:, u, pw[i] :], msc[:, pw[i] :],
                                        AF.Exp,
                                    )
                                exs[i] = ex
                                if i > 0:
                                    _den_pv8(i - 1)
                            _den_pv8(n_step - 1)
                        else:
                            n_k = len(kts)
                            exs = [None] * n_k

                            def _den_pv(i):
                                kt = kts[i]
                                w0 = wlo[kt]
                                nc.tensor.matmul(
                                    den[:, w0:], ones[:], exs[i][:, w0:],
                                    start=(i == 0), stop=(i == n_k - 1),
                                )
                                nc.tensor.matmul(
                                    pa[:, w0:],
                                    vNb[:, kt, ts(h, HD)],
                                    exs[i][:, w0:],
                                    start=(i == 0),
                                    stop=(i == n_k - 1),
                                )

                            for i, kt in enumerate(kts):
                                msc = msc_for(kt, wlo[kt])
                                ex = expp.tile([P, QW], BF16, tag="expb")
                                nc.scalar.activation(
                                    ex[:, wlo[kt] :], msc[:, wlo[kt] :], AF.Exp
                                )
                                exs[i] = ex
                                if i > 0:
                                    _den_pv(i - 1)
                            _den_pv(n_k - 1)
                        rec = stats.tile([P, QW], F32, tag="rec")
                        nc.vector.reciprocal_approx_fast(rec[:], den[:])
                        nc.vector.tensor_mul(attnT[:, h, :], pa[:], rec[:])
                        # ship this head's block immediately
                        if b == 1 and j >= 2:
                            nc.sync.dma_start(
                                a2a_in[1 + j].ap()[:, h, :, :].rearrange(
                                    "s p t -> p s t"
                                ),
                                attnT[:, h, :],
                            )
                        else:
                            nc.sync.dma_start(
                                a2a_in[g].ap()[
                                    4 * r : 4 * r + 4, h, :, :
                                ].rearrange("s p t -> p s t"),
                                attnT[:, h, :],
                            )

                    if cs_next is not None:
                        rinv_next = stats_fin(cs_next, scaled=True)

                    if b == 1 and j == 1:
                        # wo for chunks 0+1, two q-tiles after A2A-1 issue
                        emit_wo_pair(0)

                    if b == 1 and j >= 2:
                        emit_a2a(1 + j)
                    elif r == 1:
                        emit_a2a(g)

            # ---- pre-start up on cols 0:256 (ready after wo pair 0) to
            # fill the final-A2A wait with PE work ----
            with tc.tile_pool(name="w1pre", bufs=2) as w1prep:
                for ch in range(NPRE):
                    w1c = w1prep.tile(
                        [P, C, QW], BF16, tag="w1pre", name=f"w1pre{ch}"
                    )
                    (nc.gpsimd if ch % 2 else nc.sync).dma_start(w1c[:], w1h[ch])
                    for hti in range(HTPC):
                        pu = psum.tile([P, QW // 2], F32, tag="mm", bufs=3)
                        for c in range(C):
                            nc.tensor.matmul(
                                pu[:],
                                w1c[:, c, ts(hti, P)],
                                hf[:, c, 0 : QW // 2],
                                start=(c == 0),
                                stop=(c == C - 1),
                            )
                        # relu on DVE, not ACT: ACT is the attention exp
                        # critical path while this filler work runs
                        nc.vector.tensor_scalar_max(
                            up_pre[:, ch * HTPC + hti, :], pu[:], 0.0
                        )

                # ---- wo for chunks 2+3 (tail, after A2A-3a/3b) ----
                emit_wo_pair(1)

        # ---------------- FFN phase (data-parallel, 512 tokens/core) --------
        with tc.tile_pool(name="w1p", bufs=2) as w1p, \
             tc.tile_pool(name="w2p", bufs=3) as w2p, \
             tc.tile_pool(name="up", bufs=1) as upp, \
             tc.tile_pool(name="oev", bufs=3) as oevp:
            up_sb = upp.tile([P, HIDC, QW], BF16, tag="up")

            # rmsnorm2 stats (consumed only at the down outputs)
            cs = psum.tile([P, QW], F32, tag="mm", bufs=3)
            for cp in range(C // 2):
                sq = sqp.tile([P, 2, QW], F8, tag="sq")
                for i in range(2):
                    nc.vector.tensor_mul(
                        sq[:, i, :], hf[:, 2 * cp + i, :], hf[:, 2 * cp + i, :]
                    )
                nc.tensor.matmul(
                    cs[:], ones2[:], sq[:],
                    start=(cp == 0), stop=(cp == C // 2 - 1),
                    perf_mode=DR,
                )
            rms2 = stats.tile([P, QW], F32, tag="rms")
            nc.scalar.activation(
                rms2[:], cs[:], AF.Sqrt, bias=eps_p1[:], scale=1.0 / D
            )
            r2 = stats.tile([P, QW], F32, tag="rinv")
            nc.vector.reciprocal_approx_fast(r2[:], rms2[:])

            # pre-started halves: copy into up_sb
            for idx in range(NPRE * HTPC):
                nc.vector.tensor_copy(
                    up_sb[:, idx, 0 : QW // 2], up_pre[:, idx, :]
                )

            # ---- up: single w1 stream, N=512 (relu defers rinv2);
            # pre-started chunks only need their remaining columns ----
            for ch in range(W1CH):
                w1c = w1p.tile([P, C, QW], BF16, tag="w1", name=f"w1_{ch}")
                (nc.scalar if ch % 2 else nc.sync).dma_start(w1c[:], w1h[ch])
                cols = slice(QW // 2, QW) if ch < NPRE else slice(0, QW)
                for hti in range(HTPC):
                    pu = psum.tile([P, QW], F32, tag="mm", bufs=3)
                    pw = cols.stop - cols.start
                    for c in range(C):
                        nc.tensor.matmul(
                            pu[:, :pw],
                            w1c[:, c, ts(hti, P)],
                            hf[:, c, cols],
                            start=(c == 0),
                            stop=(c == C - 1),
                        )
                    nc.scalar.activation(
                        up_sb[:, ch * HTPC + hti, cols], pu[:, :pw], AF.Relu
                    )

            # ---- down ----
            for ot in range(C):
                w2c = w2p.tile([P, HIDC, P], BF16, tag="w2", name=f"w2_{ot}")
                (nc.scalar if ot % 2 else nc.sync).dma_start(w2c[:], w2h[ot])
                pd = psum.tile([P, QW], F32, tag="mm", bufs=3)
                for hc in range(HIDC):
                    nc.tensor.matmul(
                        pd[:],
                        w2c[:, hc, :],
                        up_sb[:, hc, :],
                        start=(hc == 0),
                        stop=(hc == HIDC - 1),
                    )
                dn = oevp.tile([P, QW], F32, tag="dn")
                nc.vector.tensor_mul(dn[:], pd[:], r2[:])
                oev = oevp.tile([P, QW], F32, tag="oev")
                nc.vector.tensor_add(oev[:], hf[:, ot, :], dn[:])
                (nc.scalar if ot % 2 else nc.sync).dma_start(out[ot], oev[:])

    nc.compile()
    return nc


_CACHE = {}
LAST_RESULT = None


def _get_program(B, S, D, H, HID, mask_table, n_mask, mask_key):
    key = (B, S, D, H, HID, mask_key)
    if key not in _CACHE:
        _CACHE[key] = build_program(B, S, D, H, HID, mask_table, n_mask)
    return _CACHE[key]


def _core_tokens(core, S):
    """(token start, length) of this core's output slices, in the order
    they appear in the per-core 512 token columns. Chunks 0-2 are
    128-token slices; the split last A2A gives two 64-token slices."""
    toks = []
    for g in range(3):
        b = g // 2
        j = 2 * (g % 2) + core // 4
        toks.append((S * b + QW * j + SL * (core % 4), SL))
    toks.append((S + QW * 2 + (SL // 2) * core, SL // 2))
    toks.append((S + QW * 3 + (SL // 2) * core, SL // 2))
    return toks


def kernel(x, mask, wq, wk, wv, wo, w1, w2, attn_norm_w, ffn_norm_w):
    x = np.asarray(x, dtype=np.float32)
    mask = np.asarray(mask, dtype=np.float32)
    wq, wk, wv, wo = (np.asarray(a, dtype=np.float32) for a in (wq, wk, wv, wo))
    w1, w2 = np.asarray(w1, dtype=np.float32), np.asarray(w2, dtype=np.float32)
    attn_norm_w = np.asarray(attn_norm_w, dtype=np.float32)
    ffn_norm_w = np.asarray(ffn_norm_w, dtype=np.float32)

    B, S, D = x.shape
    H = D // 128  # HD is fixed at 128 (= SBUF partition count)
    HID = w1.shape[0]
    HD = D // H
    HPC = H // N_CORES
    C = D // P
    CQ = C // 4
    S_TILES = S // QW
    HIDC = HID // P
    W1CH = 16

    mask_table, mtiles_np = _classify_mask(
        np.broadcast_to(mask, (1, 1, S, S))[0, 0], S
    )
    mask_key = hash(tuple(sorted((k, str(v)) for k, v in mask_table.items())))
    nc = _get_program(B, S, D, H, HID, mask_table, len(mtiles_np), mask_key)

    # ---- host-side prep ----
    # x tiles: [b, j, p, c, t] = x[b, j*QW + t, c*128 + p]
    xt = x.reshape(B, S_TILES, QW, C, P).transpose(0, 1, 4, 3, 2)
    xbq = np.ascontiguousarray(
        xt[:, 0].reshape(B, P, 4, CQ, QW).transpose(0, 2, 1, 3, 4)
    ).astype(BF16_NP)
    xb8h = np.ascontiguousarray(xt[:, 1:]).astype(F8_NP)

    wq_f = (wq * attn_norm_w[None, :]) / np.sqrt(HD)
    wk_f = wk * attn_norm_w[None, :]
    wv_f = wv * attn_norm_w[None, :]
    w1_f = w1 * ffn_norm_w[None, :]

    # wo3h[ot, p, oc, o] = wo[ot*128 + o, oc*128 + p]
    wo3host = np.ascontiguousarray(
        wo.reshape(C, P, H, P).transpose(0, 3, 2, 1)
    ).astype(BF16_NP)
    # w1h[ch, p, c, o] = w1_f[hid = ch*512 + o, d = c*128 + p]
    w1host = np.ascontiguousarray(
        w1_f.reshape(W1CH, QW, C, P).transpose(0, 3, 2, 1)
    ).astype(BF16_NP)
    # w2h[ot, p, hc, o] = w2[d_out = ot*128 + o, hid = hc*128 + p]
    w2host = np.ascontiguousarray(
        w2.reshape(C, P, HIDC, P).transpose(0, 3, 2, 1)
    ).astype(BF16_NP)

    xf = x.reshape(B * S, D)
    in_maps = []
    for core in range(N_CORES):
        hs = slice(core * HPC * HD, (core + 1) * HPC * HD)
        qsb = np.ascontiguousarray(wq_f[hs].T).reshape(C, P, HPC * HD)
        ksb = np.ascontiguousarray(wk_f[hs].T).reshape(C, P, HPC * HD)
        vsb = np.ascontiguousarray(wv_f[hs].T).reshape(C, P, HPC * HD)
        # xres[gp, p, cc, t]: 256 tokens per wo pair
        xr = np.empty((2, P, C, 2 * SL), dtype=np.float32)
        col = 0
        for tok0, ln in _core_tokens(core, S):
            gp, c0 = col // (2 * SL), col % (2 * SL)
            xr[gp, :, :, c0 : c0 + ln] = (
                xf[tok0 : tok0 + ln, :].T.reshape(C, P, ln).transpose(1, 0, 2)
            )
            col += ln
        m = {
            "xbq": xbq,
            "xb8h": xb8h,
            "xres": xr.astype(F16_NP),
            "wqq": qsb.reshape(4, CQ, P, HPC * HD).astype(BF16_NP),
            "wkq": ksb.reshape(4, CQ, P, HPC * HD).astype(BF16_NP),
            "wvb": vsb.astype(BF16_NP),
            "wq8": (qsb * SW).astype(F8_NP),
            "wk8": (ksb * SW).astype(F8_NP),
            "wv8": (vsb * SW).astype(F8_NP),
            "wo3h": wo3host,
            "w1h": w1host,
            "w2h": w2host,
        }
        if len(mtiles_np):
            m["mk"] = np.stack(mtiles_np).astype(BF16_NP)
        in_maps.append(m)

    trace = os.environ.get("KTRACE", "0") == "1"
    res = run_bass_kernel_spmd(nc, in_maps, list(range(N_CORES)), trace=trace)
    global LAST_RESULT
    LAST_RESULT = res

    full = np.empty((B * S, D), dtype=np.float32)
    for core in range(N_CORES):
        o = res.results[core]["out"].reshape(D, QW)
        col = 0
        for tok0, ln in _core_tokens(core, S):
            full[tok0 : tok0 + ln, :] = o[:, col : col + ln].T
            col += ln
    return np.ascontiguousarray(full.reshape(B, S, D))
